# revision 30
# baseline (speedup 1.0000x reference)
"""Trainium2 Bass kernel for nn_AugmentedODE (B=64, N=P=512), 8-core data parallel.

Per batch the reference computes (7 matmuls of 512^3):
    Omega   = 0.5*(A - A^T)
    du      = u @ Omega + G - u @ (u^T G)
    S       = lam @ G^T
    dlam    = lam @ A + (S + S^T) @ u

Restructured to 5 fp32r matmuls + 3 PE transpose sets per batch:
    UTG = u^T G                      (native:   lhsT=u,    rhs=G)
    W   = 0.5*(A - A^T) - UTG        (DVE; A^T via fp32r PE transpose, from PSUM)
    du  = u @ W + G                  (lhsT=u^T, rhs=W; +G fused in PSUM->SBUF add)
    S   = lam @ G^T                  (lhsT=lam^T, rhs=G^T via fp32r PE transpose)
    C   = S + S^T                    (S^T via PE transpose accumulated into S's PSUM)
    dlam= lam @ A + C @ u            (8 matmuls accumulated into one PSUM group;
                                      C is symmetric so native layout works)

u^T / lam^T are pre-transposed on the host (pure data movement; lam natural is
never needed, so lam^T costs no extra DMA, and u^T trades 1MB of DMA for 16 PE
transposes).  Streaming more host-transposed copies (A^T, G^T) was measured
slower: the per-core HBM path sustains only ~260-280 GB/s in-kernel, so the
7MB/batch of this config is the sweet spot against ~182us of PE work.
"""
import numpy as np

import concourse.bass as bass
import concourse.mybir as mybir
import concourse.tile as tile
from concourse import bacc
from concourse.bass_utils import run_bass_kernel_spmd
from concourse.masks import make_identity

F32 = mybir.dt.float32
F32R = mybir.dt.float32r
AOP = mybir.AluOpType

B, N, P = 64, 512, 512
NCORES = 8
BLOC = B // NCORES          # batches per core
KB = 4                      # 512 = 4 k-blocks of 128
CH = 4                      # 4 output chunks of 128 rows


def _build_nc():
    nc = bacc.Bacc("TRN2", target_bir_lowering=False, debug=False,
                   num_devices=NCORES)

    d_u = nc.declare_dram_parameter("u", [BLOC, N, P], F32R, isOutput=False)
    d_ut = nc.declare_dram_parameter("ut", [BLOC, P, N], F32R, isOutput=False)
    d_g = nc.declare_dram_parameter("g", [BLOC, N, P], F32R, isOutput=False)
    d_a = nc.declare_dram_parameter("a", [BLOC, P, P], F32R, isOutput=False)
    d_lamt = nc.declare_dram_parameter("lamt", [BLOC, P, N], F32R, isOutput=False)
    d_du = nc.declare_dram_parameter("du", [BLOC, N, P], F32, isOutput=True)
    d_dlam = nc.declare_dram_parameter("dlam", [BLOC, N, P], F32, isOutput=True)

    with tile.TileContext(nc) as tc:
        with (
            tc.tile_pool(name="const", bufs=1) as constp,
            tc.tile_pool(name="ins", bufs=2) as insp,
            tc.tile_pool(name="mid", bufs=1) as midp,
            tc.tile_pool(name="outs", bufs=2) as outsp,
            tc.tile_pool(name="psum", bufs=8, space="PSUM") as psum,
        ):
            ident = constp.tile([128, 128], F32)
            make_identity(nc, ident[:])
            identr = constp.tile([128, 128], F32R)
            nc.vector.tensor_copy(identr[:], ident[:])

            # HAM warm-up: ~5us of dummy matmuls during the head DMA wait so
            # the first real batch runs at 2.4GHz instead of the cold 1.2GHz
            warm_ps = psum.tile([128, 512], F32, tag="ps")
            wsrc = constp.tile([128, 512], F32R)
            nc.gpsimd.memset(wsrc[:].bitcast(F32), 0.0)
            for i in range(12):
                nc.tensor.matmul(warm_ps[:], identr[:], wsrc[:],
                                 start=True, stop=True)

            for b in range(BLOC):
                u_sb = insp.tile([128, KB, P], F32R, tag="u")
                ut_sb = insp.tile([128, KB, N], F32R, tag="ut")
                g_sb = insp.tile([128, KB, P], F32R, tag="g")
                a_sb = insp.tile([128, KB, P], F32R, tag="a")
                lamt_sb = insp.tile([128, KB, N], F32R, tag="lamt")
                # issue order ~ consumption order (g/a feed the PE transposes first)
                if b == 0:
                    g_r = d_g[b].rearrange("(k p) c -> p k c", p=128)
                    a_r = d_a[b].rearrange("(k p) c -> p k c", p=128)
                    nc.sync.dma_start(g_sb[:, 0:2], g_r[:, 0:2])
                    nc.scalar.dma_start(g_sb[:, 2:4], g_r[:, 2:4])
                    nc.sync.dma_start(a_sb[:, 0:2], a_r[:, 0:2])
                    nc.scalar.dma_start(a_sb[:, 2:4], a_r[:, 2:4])
                else:
                    nc.sync.dma_start(g_sb[:], d_g[b].rearrange("(k p) c -> p k c", p=128))
                    nc.sync.dma_start(a_sb[:], d_a[b].rearrange("(k p) c -> p k c", p=128))
                nc.sync.dma_start(u_sb[:], d_u[b].rearrange("(k p) c -> p k c", p=128))
                nc.sync.dma_start(lamt_sb[:], d_lamt[b].rearrange("(k p) c -> p k c", p=128))
                nc.sync.dma_start(ut_sb[:], d_ut[b].rearrange("(k p) c -> p k c", p=128))

                # ---- Gt via PE transpose: Gt[r][p, 128c:] = G[c-block, 128r:].T ----
                # fp32r transpose mode: 1.5 cycles/row vs 2.0 for fp32
                gt_sb = midp.tile([128, KB, N], F32R, tag="gt", bufs=2)
                for r in range(CH):
                    ps = psum.tile([128, P], F32R, tag="ps")
                    for c in range(KB):
                        nc.tensor.transpose(
                            ps[:, c * 128:(c + 1) * 128],
                            g_sb[:, c, r * 128:(r + 1) * 128],
                            identr[:],
                        )
                    nc.scalar.copy(gt_sb[:, r, :], ps[:])

                # ---- At via PE transpose (stays in PSUM, consumed by DVE) ----
                at_ps = []
                for r in range(CH):
                    ps = psum.tile([128, P], F32R, tag="ps")
                    for c in range(KB):
                        nc.tensor.transpose(
                            ps[:, c * 128:(c + 1) * 128],
                            a_sb[:, c, r * 128:(r + 1) * 128],
                            identr[:],
                        )
                    at_ps.append(ps)

                # ---- M1: UTG = u^T G ; W = 0.5*(A - At) - UTG (DVE) ----
                w1_sb = midp.tile([128, KB, P], F32, tag="w1")
                w_sb = midp.tile([128, KB, P], F32R, tag="w", bufs=2)
                for r in range(CH):
                    utg = psum.tile([128, P], F32, tag="ps")
                    for k in range(KB):
                        nc.tensor.matmul(utg[:], u_sb[:, k, r * 128:(r + 1) * 128],
                                         g_sb[:, k, :], start=(k == 0), stop=(k == KB - 1))
                    nc.vector.tensor_tensor(w1_sb[:, r, :], a_sb[:, r, :].bitcast(F32),
                                            at_ps[r][:].bitcast(F32), AOP.subtract)
                    nc.vector.scalar_tensor_tensor(w_sb[:, r, :], w1_sb[:, r, :], 0.5,
                                                   utg[:], AOP.mult, AOP.subtract)

                # ---- M5: S = lam @ G^T (group left open for S^T accumulation) ----
                s_ps = []
                s_sb = midp.tile([128, KB, N], F32, tag="s")
                for r in range(CH):
                    ps = psum.tile([128, N], F32, tag="ps")
                    for k in range(KB):
                        nc.tensor.matmul(ps[:], lamt_sb[:, k, r * 128:(r + 1) * 128],
                                         gt_sb[:, k, :], start=(k == 0), stop=False)
                    nc.scalar.copy(s_sb[:, r, :], ps[:])
                    s_ps.append(ps)

                # ---- M23: du = u @ W + G ----
                du_sb = outsp.tile([128, KB, P], F32, tag="du")
                for r in range(CH):
                    ps = psum.tile([128, P], F32, tag="ps")
                    for k in range(KB):
                        nc.tensor.matmul(ps[:], ut_sb[:, k, r * 128:(r + 1) * 128],
                                         w_sb[:, k, :], start=(k == 0), stop=(k == KB - 1))
                    nc.vector.tensor_tensor(du_sb[:, r, :], ps[:],
                                            g_sb[:, r, :].bitcast(F32), AOP.add)
                nc.sync.dma_start(d_du[b].rearrange("(k p) c -> p k c", p=128), du_sb[:])

                # ---- S^T accumulated into S's PSUM -> C = S + S^T ----
                coup_sb = midp.tile([128, KB, N], F32R, tag="coup")
                for r in range(CH):
                    for c in range(KB):
                        nc.tensor.matmul(
                            s_ps[r][:, c * 128:(c + 1) * 128],
                            s_sb[:, c, r * 128:(r + 1) * 128],
                            ident[:],
                            is_transpose=True,
                            start=False, stop=(c == KB - 1),
                        )
                for r in range(CH):
                    nc.vector.tensor_copy(coup_sb[:, r, :], s_ps[r][:])

                # ---- M4+M7: dlam = lam @ A + C @ u ----
                # all M4 groups first: their 16 matmuls hide the DVE coupling
                # copies that M7 needs
                dlam_sb = outsp.tile([128, KB, P], F32, tag="dlam")
                dlam_ps = []
                for r in range(CH):
                    ps = psum.tile([128, P], F32, tag="ps")
                    for k in range(KB):
                        nc.tensor.matmul(ps[:], lamt_sb[:, k, r * 128:(r + 1) * 128],
                                         a_sb[:, k, :], start=(k == 0), stop=False)
                    dlam_ps.append(ps)
                for r in range(CH):
                    ps = dlam_ps[r]
                    for k in range(KB):
                        nc.tensor.matmul(ps[:], coup_sb[:, k, r * 128:(r + 1) * 128],
                                         u_sb[:, k, :], start=False, stop=(k == KB - 1))
                    nc.scalar.copy(dlam_sb[:, r, :], ps[:])
                    if b == BLOC - 1:
                        # tail: overlap the last batch's output with its copies
                        nc.scalar.dma_start(
                            d_dlam[b].rearrange("(k p) c -> p k c", p=128)[:, r],
                            dlam_sb[:, r, :])
                if b < BLOC - 1:
                    nc.scalar.dma_start(d_dlam[b].rearrange("(k p) c -> p k c", p=128),
                                        dlam_sb[:])

    nc.compile()
    return nc


_NC = None


def _make_in_maps(u, lam, A, G):
    u = np.ascontiguousarray(u, dtype=np.float32)
    lam = np.ascontiguousarray(lam, dtype=np.float32)
    A = np.ascontiguousarray(A, dtype=np.float32)
    G = np.ascontiguousarray(G, dtype=np.float32)
    ut = np.ascontiguousarray(np.swapaxes(u, 1, 2))
    lamt = np.ascontiguousarray(np.swapaxes(lam, 1, 2))

    in_maps = []
    for c in range(NCORES):
        sl = slice(c * BLOC, (c + 1) * BLOC)
        in_maps.append({
            "u": u[sl], "ut": ut[sl], "g": G[sl], "a": A[sl], "lamt": lamt[sl],
        })
    return in_maps


def kernel(u, lam, A, G, t=None, **_ignored):
    global _NC
    if _NC is None:
        _NC = _build_nc()
    nc = _NC

    in_maps = _make_in_maps(u, lam, A, G)
    res = run_bass_kernel_spmd(nc, in_maps, list(range(NCORES)))
    du = np.concatenate([res.results[c]["du"] for c in range(NCORES)], axis=0)
    dlam = np.concatenate([res.results[c]["dlam"] for c in range(NCORES)], axis=0)
    return du, dlam
